# revision 41
# baseline (speedup 1.0000x reference)
"""Trainium2 Bass kernel for nn_Attention_73254962200646.

Reference computation (per batch element b, all shapes hardcoded):
  qkv = conv3x3(x, W_qkv, pad=1)            x:[8,512,32,32], W_qkv:[1536,512,3,3]
  q,k,v -> [g=8 heads, n=1024, d=64]
  attn  = (q @ k^T) / (|q| |k| + eps)       cosine-similarity attention
  out   = attn @ v -> [512,32,32]
  out   = conv1x1(out, W_out); BatchNorm2d (batch stats); ReLU

Distribution: data-parallel over batch B=8 across the 8 NeuronCores (one
image per core). The only collective is a 4KB AllReduce of the BatchNorm
partial sums.

The conv3x3 uses Winograd F(2x2,3x3): 2.25x fewer PE MACs than direct
(768 matmuls of N=256 vs 864 of N=512). Input transforms are DVE adds
kept in 2x-packed bf16 mode (host pre-splits the image into even/odd +
shifted column planes so every operand is a step-1 aligned slice, and
the H-pass splits tile rows by parity so its two operands never overlap
- overlapping or in-place DVE operands run ~4x slow). Per-position
matmuls accumulate 4-cin chains in fp32 PSUM, 4 positions per
[128,1024] bank pair; the inverse transform (passA/passB) runs on
scalar copies + fresh-destination DVE adds woven into the next block's
matmuls. All pixel indexing downstream of the conv is in Winograd
quadrant order (d,e,ty,tx); attention and BN are pixel-permutation-
invariant and the host undoes the permutation for free after the
gather. Norm broadcast matmuls are bf16 (4x cheaper than fp32 PE
mode); the final BN+ReLU writes bf16 and streams out on 3 DMA queues.
"""

import numpy as np
import ml_dtypes

import concourse.tile as tile
import concourse.mybir as mybir
from concourse import bacc, bass_utils

BF = ml_dtypes.bfloat16
SMOOTH = 1e-4
BN_EPS = 1e-5
NCORES = 8

_NC = None
LAST_RESULT = None


def _build():
    f32 = mybir.dt.float32
    bf = mybir.dt.bfloat16
    AF = mybir.ActivationFunctionType
    ALU = mybir.AluOpType

    nc = bacc.Bacc("TRN2", target_bir_lowering=False, debug=False,
                   num_devices=NCORES)
    # xq: [128 cin, 4 cb, 4 kind(e,o,es,os), 34 h, 18 w] bf16
    xq_d = nc.dram_tensor("xq", [128, 4, 4, 34, 18], bf,
                          kind="ExternalInput").ap()
    # uw: [12 coutblk, 4 cinblk, 128 cin, 16 pos, 128 cout] bf16
    uw = nc.dram_tensor("uw", [12, 4, 128, 16, 128], bf,
                        kind="ExternalInput").ap()
    wo = nc.dram_tensor("wo", [4, 128, 512], bf, kind="ExternalInput").ap()
    gb = nc.dram_tensor("gb", [128, 8], f32, kind="ExternalInput").ap()
    ones2 = nc.dram_tensor("ones2", [128, 2], bf, kind="ExternalInput").ap()
    sel2 = nc.dram_tensor("sel2", [2, 128], bf, kind="ExternalInput").ap()
    ident = nc.dram_tensor("ident", [128, 128], bf, kind="ExternalInput").ap()
    out = nc.dram_tensor("out", [512, 1024], bf, kind="ExternalOutput").ap()

    with tile.TileContext(nc) as tc:
        with tc.tile_pool(name="sb", bufs=1) as sb, \
             tc.tile_pool(name="tp", bufs=2) as tp, \
             tc.tile_pool(name="ps", bufs=4, space="PSUM") as ps, \
             tc.tile_pool(name="dram", bufs=1, space="DRAM") as dram:

            xqt = sb.tile([128, 4, 4, 34, 18], bf, tag="xqt")
            # W-pass out: [128, 4 b, 4 cb, 2 eo, 18 hh, 16 tx] (hh row 17
            # unused; even hh count lets the H-pass split ty by parity so
            # its two operands never overlap - overlapping DVE operands
            # run ~4x slow)
            wv = sb.tile([128, 4, 4, 2, 18, 16], bf, tag="wv")
            # V split per a-quad: 4 tiles [128 cin, 4 cb, 4 b, 16 ty,
            # 16 tx] so the first conv matmuls depend only on their own
            # quad's H-pass ops, not all 32 of them
            vvq = [sb.tile([128, 4, 4, 16, 16], bf, tag=f"vv{a}",
                           name=f"vvq{a}")
                   for a in range(4)]
            identt = sb.tile([128, 128], bf, tag="identt")
            wot = sb.tile([128, 4, 512], bf, tag="wot")
            gbt = sb.tile([128, 8], f32, tag="gbt")
            ones2t = sb.tile([128, 2], bf, tag="ones2t")
            sel2t = sb.tile([2, 128], bf, tag="sel2t")
            qhat = sb.tile([128, 4, 1024], bf, tag="qhat")
            khat = sb.tile([128, 4, 1024], bf, tag="khat")
            vT = sb.tile([128, 8, 512], bf, tag="vT")
            att = sb.tile([128, 4, 1024], bf, tag="att")
            yt = sb.tile([128, 4, 1024], f32, tag="yt")
            part = sb.tile([128, 16], f32, tag="part")
            stats = sb.tile([128, 16], f32, tag="stats")
            epst = sb.tile([128, 1], f32, tag="epst")
            smt = sb.tile([2, 1], f32, tag="smt")

            # startup DMAs; sync queue reserved for the U weight stream
            nc.scalar.dma_start(xqt[:, 0], xq_d[:, 0])
            nc.gpsimd.dma_start(xqt[:, 1], xq_d[:, 1])
            nc.scalar.dma_start(xqt[:, 2], xq_d[:, 2])
            nc.gpsimd.dma_start(xqt[:, 3], xq_d[:, 3])
            nc.gpsimd.dma_start(identt[:], ident)
            nc.gpsimd.dma_start(ones2t[:], ones2)
            nc.gpsimd.dma_start(sel2t[:], sel2)
            for cb in range(4):
                nc.gpsimd.dma_start(wot[:, cb], wo[cb])
            nc.gpsimd.dma_start(gbt[:], gb)
            nc.vector.memset(epst[:], BN_EPS)
            nc.vector.memset(smt[:], SMOOTH)

            def emit_warm_ar():
                warm_in = dram.tile([1, 8], f32, name="warm_in")
                warm_out = dram.tile([1, 8], f32, name="warm_out")
                warm_sb = sb.tile([1, 8], f32, tag="warm_sb")
                nc.vector.memset(warm_sb[:], 0.0)
                nc.gpsimd.dma_start(warm_in[:], warm_sb[:])
                nc.gpsimd.collective_compute(
                    "AllReduce", ALU.add,
                    ins=[warm_in[:].opt()], outs=[warm_out[:].opt()],
                    replica_groups=[list(range(NCORES))])

            # ---- input transform ----
            def emit_transform():
                # W-pass, per cb: shapes [17 hh, 2 eo, 16] -> wv[:, b, cb]
                for cb in range(4):
                    e = xqt[:, cb, 0].rearrange("p (hh eo) w -> p hh eo w",
                                                eo=2)
                    o = xqt[:, cb, 1].rearrange("p (hh eo) w -> p hh eo w",
                                                eo=2)
                    es = xqt[:, cb, 2].rearrange("p (hh eo) w -> p hh eo w",
                                                 eo=2)
                    os_ = xqt[:, cb, 3].rearrange("p (hh eo) w -> p hh eo w",
                                                  eo=2)
                    dst = wv[:, :, cb].rearrange("p b eo hh w -> p b hh eo w")
                    nc.vector.tensor_sub(dst[:, 0, 0:17], e[:, 0:17, :, 0:16],
                                         es[:, 0:17, :, 0:16])
                    nc.vector.tensor_add(dst[:, 1, 0:17], o[:, 0:17, :, 0:16],
                                         es[:, 0:17, :, 0:16])
                    nc.vector.tensor_sub(dst[:, 2, 0:17], es[:, 0:17, :, 0:16],
                                         o[:, 0:17, :, 0:16])
                    nc.vector.tensor_sub(dst[:, 3, 0:17], o[:, 0:17, :, 0:16],
                                         os_[:, 0:17, :, 0:16])
                    yield None
                # H-pass per (a, b), cb-batched, ty split by parity so the
                # two source row sets never overlap. wv hh viewed as
                # (t2 two): hh = 2*t2 + two.
                for p in range(16):
                    a, b = p // 4, p % 4
                    wve = wv[:, b, :, 0].rearrange(
                        "p c (t2 two) w -> p c t2 two w", two=2)
                    wvo = wv[:, b, :, 1].rearrange(
                        "p c (t2 two) w -> p c t2 two w", two=2)
                    vvr = vvq[a][:, :, b].rearrange(
                        "p c (t2 two) w -> p c t2 two w", two=2)
                    # rows: ee[ty]=wve[t2,two] with ty=2*t2+two etc.
                    for par in (0, 1):
                        if par == 0:   # ty even: t2 0..7 two=0; ty+1 odd
                            ee = wve[:, :, 0:8, 0]
                            es_ = wve[:, :, 0:8, 1]
                            oe = wvo[:, :, 0:8, 0]
                            os2 = wvo[:, :, 0:8, 1]
                        else:          # ty odd: two=1; ty+1 = even t2+1
                            ee = wve[:, :, 0:8, 1]
                            es_ = wve[:, :, 1:9, 0]
                            oe = wvo[:, :, 0:8, 1]
                            os2 = wvo[:, :, 1:9, 0]
                        dstv = vvr[:, :, :, par]
                        if a == 0:
                            nc.vector.tensor_sub(dstv, ee, es_)
                        elif a == 1:
                            nc.vector.tensor_add(dstv, oe, es_)
                        elif a == 2:
                            nc.vector.tensor_sub(dstv, es_, oe)
                        else:
                            nc.vector.tensor_sub(dstv, oe, os2)
                    if p % 2 == 1:
                        yield None

            def conv_gen(cob):
                """Winograd conv for cout block cob: 16 positions x 4 cin
                chains of N=256 into 1-bank bf16 PSUM quads; passA woven in
                so quads free early. Yields ~1us chunks; returns raw tile
                via first yield."""
                uwt = tp.tile([128, 4, 16, 128], bf, tag="uw", bufs=2,
                              name=f"uw{cob}")
                for cb in range(4):
                    nc.sync.dma_start(uwt[:, cb], uw[cob, cb])
                s1 = tp.tile([128, 1024], bf, tag="s1", bufs=2,
                             name=f"s1_{cob}")
                s2 = tp.tile([128, 1024], bf, tag="s2", bufs=2,
                             name=f"s2_{cob}")
                y1 = tp.tile([128, 4, 1024], bf, tag="y1", bufs=1,
                             name=f"y1_{cob}")
                raw = tp.tile([128, 1024], bf, tag="raw", bufs=3,
                              name=f"raw{cob}")
                yield raw
                # 8 one-bank half-quad slots: slot s holds positions
                # p = 4a + 2h + i for a = s//2, h = s%2, i in {0,1}.
                # passA consumers are emitted immediately after each slot
                # fills so slot n is released before slot n+2's request
                # (convp ring bufs=2 -> conv holds only 2 PSUM banks).
                # s1 = copy of M(a=0), s2 = copy of M(a=1); y1 lanes:
                # 0 = t1 (M0+M1), 1 = y10, 2 = v2 (M1-M2), 3 = y11.
                for s in range(8):
                    a, h = s // 2, s % 2
                    pq = ps.tile([128, 512], f32, tag="convp", bufs=2,
                                 name=f"pq{cob}_{s}")
                    for i in range(2):
                        p = 4 * a + 2 * h + i
                        for cb in range(4):
                            nc.tensor.matmul(
                                pq[:, 256 * i:256 * (i + 1)],
                                uwt[:, cb, p, :],
                                vvq[a][:, cb, 2 * h + i],
                                start=(cb == 0), stop=(cb == 3))
                    hs = slice(512 * h, 512 * (h + 1))
                    if a == 0:
                        nc.scalar.copy(s1[:, hs], pq[:])
                    elif a == 1:
                        nc.scalar.copy(s2[:, hs], pq[:])
                        nc.vector.tensor_add(y1[:, 0, hs], s1[:, hs],
                                             s2[:, hs])
                    elif a == 2:
                        nc.vector.tensor_add(y1[:, 1, hs], y1[:, 0, hs],
                                             pq[:])
                        nc.vector.tensor_sub(y1[:, 2, hs], s2[:, hs],
                                             pq[:])
                    else:
                        nc.vector.tensor_sub(y1[:, 3, hs], y1[:, 2, hs],
                                             pq[:])
                    yield None
                # passB -> raw [128, (2d 2e 256)]; yd = [y10, y11]
                yd = y1.rearrange("p (f g) t -> p f g t", g=2)[:, :, 1]
                ydv = yd.rearrange("p d (b t) -> p d b t", b=4)
                rv = raw.rearrange("p (d e t) -> p d e t", d=2, e=2)
                wsc = tp.tile([128, 2, 2, 256], bf, tag="wsc", bufs=2,
                              name=f"wsc{cob}")
                nc.vector.tensor_add(wsc[:, :, 0], ydv[:, :, 0], ydv[:, :, 1])
                nc.vector.tensor_sub(wsc[:, :, 1], ydv[:, :, 1], ydv[:, :, 2])
                yield None
                nc.vector.tensor_add(rv[:, :, 0], wsc[:, :, 0], ydv[:, :, 2])
                nc.vector.tensor_sub(rv[:, :, 1], wsc[:, :, 1], ydv[:, :, 3])
                yield None

            def post_gen(cob, raw):
                """Per-kind epilogue on the finished conv block."""
                if cob >= 8:   # v block: PE-transpose into vT
                    m = cob - 8
                    for c2 in range(2):
                        pt = ps.tile([128, 512], bf, tag="convp", bufs=2,
                                     name=f"pt{cob}_{c2}")
                        for c in range(4):
                            j = 4 * c2 + c
                            nc.tensor.transpose(pt[:, 128 * c:128 * (c + 1)],
                                                raw[:, 128 * j:128 * (j + 1)],
                                                identt[:])
                        dstv = vT[:, 4 * c2:4 * (c2 + 1), 128 * m:128 * (m + 1)]
                        srcv = pt[:].rearrange("p (a b) -> p a b", a=4)
                        if c2 == 0:
                            nc.scalar.copy(dstv, srcv)
                        else:
                            nc.vector.tensor_copy(out=dstv, in_=srcv)
                        yield None
                else:          # q/k block: cosine norms + normalized copy
                    m = cob % 4
                    dst = qhat if cob < 4 else khat
                    nrm = tp.tile([2, 1024], f32, tag="nrm", bufs=1,
                                  name=f"nrm{cob}")
                    inv = tp.tile([2, 1024], f32, tag="inv", bufs=1,
                                  name=f"inv{cob}")
                    invb = tp.tile([2, 1024], bf, tag="invb", bufs=1,
                                   name=f"invb{cob}")
                    sq = tp.tile([128, 1024], bf, tag="sq", bufs=1,
                                 name=f"sq{cob}")
                    nc.scalar.square(sq[:, 0:512], raw[:, 0:512])
                    nc.vector.tensor_mul(sq[:, 512:1024], raw[:, 512:1024],
                                         raw[:, 512:1024])
                    yield None
                    for t in range(2):
                        pss = ps.tile([2, 512], f32, tag="convp", bufs=2,
                                      name=f"pss{cob}_{t}")
                        nc.tensor.matmul(pss[:], ones2t[:],
                                         sq[:, 512 * t:512 * (t + 1)],
                                         start=True, stop=True)
                        nc.scalar.activation(out=nrm[:, 512 * t:512 * (t + 1)],
                                             in_=pss[:], func=AF.Sqrt,
                                             bias=smt[:], scale=1.0)
                        yield None
                    nc.vector.reciprocal_approx_fast(out=inv[:], in_=nrm[:])
                    nc.scalar.copy(invb[:], inv[:])
                    yield None
                    for t in range(2):
                        pbc = ps.tile([128, 512], f32, tag="convp", bufs=2,
                                      name=f"pbc{cob}_{t}")
                        nc.tensor.matmul(pbc[:], sel2t[:],
                                         invb[:, 512 * t:512 * (t + 1)],
                                         start=True, stop=True)
                        nc.vector.tensor_mul(dst[:, m, 512 * t:512 * (t + 1)],
                                             raw[:, 512 * t:512 * (t + 1)],
                                             pbc[:])
                        yield None

            def att_gen(m):
                """Attention pair (heads 2m, 2m+1): 2 chunks per j block.
                Scores use 4 one-bank PSUM tiles per j, each released
                independently by its own evacuation copy."""
                po = ps.tile([128, 1024], f32, tag="pop", bufs=1,
                             name=f"po{m}")
                prev = None
                for j in range(8):
                    if prev is not None:
                        emit_outT(m, po, *prev)
                    pa0 = ps.tile([128, 1024], f32, tag="attp", bufs=2,
                                  name=f"pa0_{m}_{j}")
                    if m == 3 or (m == 2 and j >= 6):
                        # post-conv js: borrow the idle convp banks for
                        # pa1 so qk never stalls on slot reuse
                        pa1a = ps.tile([128, 512], f32, tag="convp", bufs=2,
                                       name=f"pa1a_{m}_{j}")
                        pa1b = ps.tile([128, 512], f32, tag="convp", bufs=2,
                                       name=f"pa1b_{m}_{j}")
                        p1 = [pa1a[:], pa1b[:]]
                    else:
                        pa1a = None
                        pa1 = ps.tile([128, 1024], f32, tag="attp", bufs=2,
                                      name=f"pa1_{m}_{j}")
                        p1 = [pa1[:, 0:512], pa1[:, 512:1024]]
                    for t in range(2):
                        nc.tensor.matmul(pa0[:, 512 * t:512 * (t + 1)],
                                         khat[0:64, m, 128 * j:128 * (j + 1)],
                                         qhat[0:64, m, 512 * t:512 * (t + 1)],
                                         start=True, stop=True)
                        nc.tensor.matmul(p1[t],
                                         khat[64:128, m, 128 * j:128 * (j + 1)],
                                         qhat[64:128, m, 512 * t:512 * (t + 1)],
                                         start=True, stop=True)
                    yield None
                    yield None   # pacing: let copies(j-1) clear the queues
                    a0 = tp.tile([128, 1024], bf, tag="attnT", bufs=6,
                                 name=f"a0_{m}_{j}")
                    a1 = tp.tile([128, 1024], bf, tag="attnT", bufs=6,
                                 name=f"a1_{m}_{j}")
                    nc.scalar.copy(a0[:], pa0[:])
                    if m == 3 or (m == 2 and j >= 6):
                        nc.vector.tensor_copy(out=a1[:, 0:512], in_=p1[0])
                        nc.vector.tensor_copy(out=a1[:, 512:1024], in_=p1[1])
                    else:
                        nc.vector.tensor_copy(out=a1[:], in_=pa1[:])
                    yield None
                    prev = (j, a0, a1)
                emit_outT(m, po, *prev)
                if m % 2 == 0:
                    nc.scalar.copy(att[:, m, :], po[:])
                else:
                    nc.vector.tensor_copy(out=att[:, m, :], in_=po[:])
                yield None

            def emit_outT(m, po, j, a0, a1):
                for t in range(2):
                    nc.tensor.matmul(po[0:64, 512 * t:512 * (t + 1)],
                                     vT[:, j, 128 * m:128 * m + 64],
                                     a0[:, 512 * t:512 * (t + 1)],
                                     start=(j == 0), stop=(j == 7),
                                     tile_position=(0, 0))
                    nc.tensor.matmul(po[64:128, 512 * t:512 * (t + 1)],
                                     vT[:, j, 128 * m + 64:128 * (m + 1)],
                                     a1[:, 512 * t:512 * (t + 1)],
                                     start=(j == 0), stop=(j == 7),
                                     tile_position=(0, 64))

            def conv1x1_gen():
                bscr = tp.tile([128, 1024], bf, tag="bscr", bufs=1,
                               name="bscr")
                for c4 in range(4):
                    py = ps.tile([128, 1024], f32, tag="attp", bufs=2,
                                 name=f"py{c4}")
                    for t in range(2):
                        for cb in range(4):
                            nc.tensor.matmul(py[:, 512 * t:512 * (t + 1)],
                                             wot[:, cb, 128 * c4:128 * (c4 + 1)],
                                             att[:, cb, 512 * t:512 * (t + 1)],
                                             start=(cb == 0), stop=(cb == 3))
                    yield None
                    nc.vector.tensor_scalar(
                        out=yt[:, c4, :], in0=py[:],
                        scalar1=1.0, scalar2=None,
                        op0=ALU.mult, op1=ALU.add,
                        accum_out=part[:, 2 * c4:2 * c4 + 1])
                    nc.scalar.activation(out=bscr[:], in_=py[:],
                                         func=AF.Square,
                                         accum_out=part[:, 8 + 2 * c4:9 + 2 * c4])
                    yield None

            def drain(g):
                if g is not None:
                    for _ in g:
                        pass

            def interleave(main, fillers, lead=1):
                """Drain `main`; after each of its chunks past `lead`,
                emit one chunk from the filler queue (round-robin).
                Unfinished fillers persist for the next block."""
                i = 0
                for _ in main:
                    i += 1
                    if i > lead:
                        while fillers:
                            g = fillers[0]
                            try:
                                next(g)
                                fillers.append(fillers.pop(0))
                                break
                            except StopIteration:
                                fillers.pop(0)

            # ---- emission plan ----
            # Block order: v first (pair readiness), then interleaved
            # q/k so attention pair m unlocks after its k block. Each
            # block's per-kind epilogue and any ready attention pair
            # weave into the next block's conv matmuls.
            # Attention pairs must run strictly one at a time: two pairs
            # interleaved on the attp/attnT rings create a cross-FIFO
            # slot-reuse cycle (a stalled matmul at the head of the PE
            # queue blocks the other pair's matmul whose copy would free
            # the first one's slot). att_runner plays queued pairs
            # sequentially; post_then_queue appends pair m only after
            # post_gen(k_m) has fully emitted (it reads khat[m]).
            att_queue = []

            def att_runner():
                i = 0
                while i < 4:
                    if i < len(att_queue):
                        yield from att_queue[i]
                        i += 1
                    else:
                        yield None

            def post_then_queue(pg, m):
                yield from pg
                att_queue.append(att_gen(m))

            drain(emit_transform())
            emit_warm_ar()
            order = [8, 9, 0, 4, 10, 1, 5, 11, 2, 6, 3, 7]
            att_after = {3: 0, 6: 1, 9: 2, 11: 3}
            fillers = [att_runner()]
            for ib, cob in enumerate(order):
                g = conv_gen(cob)
                raw = next(g)
                interleave(g, fillers, lead=1)
                pg = post_gen(cob, raw)
                if ib in att_after:
                    pg = post_then_queue(pg, att_after[ib])
                fillers.append(pg)
            # drain remaining: last k epilogue + attention pairs 2/3
            while fillers:
                g = fillers[0]
                try:
                    next(g)
                    fillers.append(fillers.pop(0))
                except StopIteration:
                    fillers.pop(0)
            drain(conv1x1_gen())

            # ---- BatchNorm: AllReduce 4KB of partial sums, then apply ----
            cin_d = dram.tile([128, 16], f32)
            cout_d = dram.tile([128, 16], f32)
            nc.gpsimd.dma_start(cin_d[:], part[:])
            nc.gpsimd.collective_compute(
                "AllReduce", ALU.add,
                ins=[cin_d[:].opt()], outs=[cout_d[:].opt()],
                replica_groups=[list(range(NCORES))])
            nc.sync.dma_start(stats[:], cout_d[:])

            var = sb.tile([128, 4], f32, tag="var")
            stdt = sb.tile([128, 4], f32, tag="stdt")
            rstd = sb.tile([128, 4], f32, tag="rstd")
            scl = sb.tile([128, 4], f32, tag="scl")
            sht = sb.tile([128, 4], f32, tag="sht")
            msq = sb.tile([128, 4], f32, tag="msq")
            tmp = sb.tile([128, 4], f32, tag="tmp")
            NINV = 1.0 / 8192.0
            msum = sb.tile([128, 4], f32, tag="msum")
            esum = sb.tile([128, 4], f32, tag="esum")
            nc.vector.tensor_scalar_mul(stats[:], stats[:], NINV)
            sr = stats[:, 0:8].rearrange("p (c t) -> p c t", t=2)
            er = stats[:, 8:16].rearrange("p (c t) -> p c t", t=2)
            nc.vector.tensor_add(msum[:], sr[:, :, 0], sr[:, :, 1])
            nc.vector.tensor_add(esum[:], er[:, :, 0], er[:, :, 1])
            mean = msum[:]
            nc.vector.tensor_mul(msq[:], mean[:], mean[:])
            nc.vector.tensor_sub(var[:], esum[:], msq[:])
            nc.scalar.activation(out=stdt[:], in_=var[:], func=AF.Sqrt,
                                 bias=epst[:], scale=1.0)
            nc.vector.reciprocal_approx_fast(out=rstd[:], in_=stdt[:])
            nc.vector.tensor_mul(scl[:], gbt[:, 0:4], rstd[:])
            nc.vector.tensor_mul(tmp[:], mean[:], scl[:])
            nc.vector.tensor_sub(sht[:], gbt[:, 4:8], tmp[:])
            out_q = [nc.sync, nc.gpsimd, nc.scalar, nc.gpsimd]
            for c4 in range(4):
                nc.scalar.activation(out=att[:, c4, :], in_=yt[:, c4, :],
                                     func=AF.Relu,
                                     scale=scl[:, c4:c4 + 1],
                                     bias=sht[:, c4:c4 + 1])
                out_q[c4].dma_start(out[128 * c4:128 * (c4 + 1), :],
                                    att[:, c4, :])

    nc.compile()
    return nc


def _prep_inputs(x, W_qkv, W_out, gamma, beta):
    x = np.asarray(x, np.float32)
    W_qkv = np.asarray(W_qkv, np.float32)
    W_out = np.asarray(W_out, np.float32)
    gamma = np.asarray(gamma, np.float32)
    beta = np.asarray(beta, np.float32)

    # x -> even/odd/shifted column planes, padded
    xs = x.reshape(8, 4, 128, 32, 32)
    xpad = np.zeros((8, 4, 128, 34, 34), np.float32)
    xpad[:, :, :, 1:33, 1:33] = xs
    xe = xpad[..., 0::2]                       # [8,4,128,34,17]
    xo = xpad[..., 1::2]
    kind = np.zeros((8, 4, 128, 4, 34, 18), np.float32)
    kind[:, :, :, 0, :, 0:17] = xe
    kind[:, :, :, 1, :, 0:17] = xo
    kind[:, :, :, 2, :, 0:16] = xe[..., 1:17]
    kind[:, :, :, 3, :, 0:16] = xo[..., 1:17]
    xq = np.ascontiguousarray(
        kind.transpose(0, 2, 1, 3, 4, 5).astype(BF))  # [8,128,4,4,34,18]

    # Winograd weight transform U = G w G^T  -> [cout, 4a, cin, 4b]
    G = np.array([[1, 0, 0], [.5, .5, .5], [.5, -.5, .5], [0, 0, 1]],
                 np.float64)
    U = np.einsum('ab,oibd,ed->oaie', G, W_qkv.astype(np.float64), G)
    U = U.astype(np.float32)                   # [1536, 4, 512, 4]
    U6 = U.reshape(12, 128, 4, 4, 128, 4)      # [cob, cout, a, cinb, cin, b]
    uwl = np.ascontiguousarray(
        U6.transpose(0, 3, 4, 2, 5, 1)         # [cob, cinb, cin, a, b, cout]
        .reshape(12, 4, 128, 16, 128).astype(BF))

    wo = np.ascontiguousarray(
        W_out[:, :, 0, 0].T.reshape(4, 128, 512).astype(BF))
    gb = np.ascontiguousarray(np.concatenate(
        [gamma.reshape(4, 128).T, beta.reshape(4, 128).T], axis=1)
        .astype(np.float32))
    p = np.arange(128)
    ones2 = np.ascontiguousarray(
        np.stack([p < 64, p >= 64], axis=1).astype(BF))
    sel2 = np.ascontiguousarray(
        np.stack([p < 64, p >= 64], axis=0).astype(BF))
    identv = np.eye(128, dtype=BF)

    common = {"uw": uwl, "wo": wo, "gb": gb,
              "ones2": ones2, "sel2": sel2, "ident": identv}
    return [{"xq": np.ascontiguousarray(xq[b]), **common}
            for b in range(8)]


def kernel(x, W_qkv, W_out, gamma, beta):
    global _NC, LAST_RESULT
    if _NC is None:
        _NC = _build()
    in_maps = _prep_inputs(x, W_qkv, W_out, gamma, beta)
    res = bass_utils.run_bass_kernel_spmd(
        _NC, in_maps, core_ids=list(range(NCORES)))
    LAST_RESULT = res
    outs = []
    for b in range(8):
        y = np.asarray(res.results[b]["out"],
                       np.float32).reshape(512, 2, 2, 16, 16)
        outs.append(np.ascontiguousarray(
            y.transpose(0, 3, 1, 4, 2)).reshape(512, 32, 32))
    return np.stack(outs).astype(np.float32)
